# revision 29
# baseline (speedup 1.0000x reference)
"""Trainium2 Bass kernel for CarlosSelfAttention (B=2, T=2048, C=1024, H=16).

Sharding: tensor-parallel over heads. 8 cores x 2 heads each. Each core
computes q/k/v projections for its 2 heads, RoPE, causal attention, and a
partial out-projection against its 128 columns of Wo. The host sums the 8
partial outputs (the TP all-reduce) and adds the output bias plus the
(v-bias @ Wo.T) correction term.

All-bf16 datapath (fp32 PSUM accumulation). Per-core layout:
  xsb  [128, 8ct, 4096]   whole input resident in SBUF, bf16
  qT/kT[128, 4096]        rows = [h0-even, h0-odd, h1-even, h1-odd] dims
  Vsb  [128, b, kt, 132]  V^T tiles: per head 66 cols = [ones, 64 dims, pad]
  S^T  [128 kpos, 1024]   psum; exp'd on ScalarE -> pt bf16, causal-trimmed
  PV   stationary = P^T [128,128] tile, moving = Vsb 66-wide; 8 accumulation
       groups packed in 2 psum banks (single bank reset at kt=0).
  norm 1/Z via per-partition tensor_scalar_mul; PE-transpose -> OT [dims,tok]
  out  y_part [4096, 1024] bf16 = OT.T @ WoT via PE.
QKV of batch 1 and out-proj of batch 0 run as fillers inside the attention
phases to keep the tensor engine dense.
"""

import numpy as np
import ml_dtypes

import concourse.bass as bass
import concourse.tile as tile
from concourse import bacc, mybir
from concourse.bass_utils import run_bass_kernel_spmd

F32 = mybir.dt.float32
BF16 = mybir.dt.bfloat16
AF = mybir.ActivationFunctionType
BF = ml_dtypes.bfloat16

B, T, C, H, HD = 2, 2048, 1024, 16, 64
NCORES = 8
TB = B * T          # 4096
QCH = 512           # q-chunk
NQC = T // QCH      # 4 q-chunks per batch
NKT = T // 128      # 16 k-tiles per batch
NCT = C // 128      # 8 contraction tiles
VW = HD + 2         # 66: [ones, 64 dims, pad] moving width per head in PV
PIPE = 2

_PROG_CACHE: dict = {}


def _emit(tc, mode, dram):
    nc = tc.nc
    from contextlib import ExitStack

    xT, wqkT, wvT, bqk, cosT, sinS, woT, y = (
        dram["xT"], dram["wqkT"], dram["wvT"], dram["bqk"], dram["cosT"],
        dram["sinS"], dram["woT"], dram["y"])
    maskT = dram.get("maskT")

    with ExitStack() as ctx:
        constp = ctx.enter_context(tc.tile_pool(name="const", bufs=1))
        pers = ctx.enter_context(tc.tile_pool(name="pers", bufs=1))
        psSp = ctx.enter_context(tc.tile_pool(name="psS", bufs=2, space="PSUM"))
        psOp = ctx.enter_context(tc.tile_pool(name="psO", bufs=1, space="PSUM"))
        auxp = ctx.enter_context(tc.tile_pool(name="aux", bufs=2, space="PSUM"))
        ptp = ctx.enter_context(tc.tile_pool(name="ptp", bufs=5))
        swpp = ctx.enter_context(tc.tile_pool(name="swp", bufs=2))
        rtp = ctx.enter_context(tc.tile_pool(name="rtp", bufs=2))
        onp = ctx.enter_context(tc.tile_pool(name="onp", bufs=6))
        smol = ctx.enter_context(tc.tile_pool(name="smol", bufs=4))
        ybp = ctx.enter_context(tc.tile_pool(name="ybp", bufs=4))
        mbp = ctx.enter_context(tc.tile_pool(name="mbp", bufs=4))

        # ---- constants (all host-pre-tiled: contiguous DMAs) ----
        wqk_sb = constp.tile([128, NCT, 256], BF16)
        nc.sync.dma_start(wqk_sb[:], wqkT[:])
        bqk_sb = constp.tile([128, 2], F32)
        nc.sync.dma_start(bqk_sb[:], bqk[:])
        # whole input resident in SBUF, one tile per 512-token chunk so the
        # first projection unit only waits for its own chunk's DMA
        xch = [pers.tile([128, NCT, 512], BF16, name=f"xch{ch}")
               for ch in range(8)]
        for ch in (0, 1, 2, 3):
            nc.sync.dma_start(xch[ch][:], xT[:, ch, :, :])
        cos_sb = constp.tile([128, T], BF16)
        nc.sync.dma_start(cos_sb[:], cosT[:])
        sin_sb = constp.tile([128, T], BF16)
        nc.sync.dma_start(sin_sb[:], sinS[:])
        wv_sb = constp.tile([128, NCT, 128], BF16)
        nc.sync.dma_start(wv_sb[:], wvT[:])
        wo_sb = constp.tile([128, C], BF16)

        def load_unit(ch):
            if ch < 8:
                nc.sync.dma_start(xch[ch][:], xT[:, ch, :, :])
            else:
                nc.sync.dma_start(wo_sb[:], woT[:])

        def xs(b, tok0, width):
            """xsb slice [128, NCT, width] for batch b tokens [tok0, tok0+width)."""
            ch, o = divmod(b * T + tok0, 512)
            assert o + width <= 512
            return xch[ch][:, :, o:o + width]
        id128 = constp.tile([128, 128], F32)
        nc.vector.memset(id128[:], 1.0)
        nc.gpsimd.affine_select(
            out=id128[:], in_=id128[:], compare_op=mybir.AluOpType.is_equal,
            fill=0.0, base=0, channel_multiplier=1, pattern=[[-1, 128]])

        # ---- persistent activations (per-chunk tiles: fine-grained deps) ----
        qTc = [[pers.tile([128, QCH], BF16, name=f"qT{b}{tch}")
                for tch in range(NQC)] for b in range(B)]
        kTc = [[pers.tile([128, QCH], BF16, name=f"kT{b}{tch}")
                for tch in range(NQC)] for b in range(B)]
        # V^T tiles [128 kpos, B, NKT, 2*VW]; ones/pad cols preset to 1.0
        Vsb = pers.tile([128, B, NKT, 2 * VW], BF16)
        nc.vector.memset(Vsb[:], 1.0)
        OT = pers.tile([128, B, T], BF16)

        # ---- unit emitters ----
        def qk_unit(b, tch, g, evict_eng):
            """q or k projection for one 512-token chunk of batch b."""
            xv = xs(b, tch * QCH, QCH)
            ps = auxp.tile([128, QCH], F32, tag="aux", name=f"qk{b}{tch}{g}")
            for ct in range(NCT):
                nc.tensor.matmul(ps[:], wqk_sb[:, ct, g * 128:(g + 1) * 128],
                                 xv[:, ct, :], start=(ct == 0),
                                 stop=(ct == NCT - 1))
            dst = (qTc if g == 0 else kTc)[b][tch][:]
            if evict_eng == "scalar":
                nc.scalar.activation(dst, ps[:], AF.Identity,
                                     bias=bqk_sb[:, g:g + 1])
            else:
                nc.vector.tensor_scalar_add(dst, ps[:], bqk_sb[:, g:g + 1])

        def v_unit(b, vt):
            """V^T for one 128-token tile of batch b (both heads)."""
            xv = xs(b, vt * 128, 128)
            ps = auxp.tile([128, QCH], F32, tag="aux", name=f"v{b}{vt}")
            for ct in range(NCT):
                nc.tensor.matmul(ps[:, 0:128], xv[:, ct, :],
                                 wv_sb[:, ct, :], start=(ct == 0),
                                 stop=(ct == NCT - 1))
            dst = Vsb[:, b, vt, :].rearrange("p (h c) -> p h c", h=2)[:, :, 1:65]
            nc.vector.tensor_copy(
                dst, ps[:, 0:128].rearrange("p (h c) -> p h c", h=2))

        def rope_unit(b, zc, tch, nm):
            """RoPE in-place on one per-chunk tile zc = (qTc|kTc)[b][tch]."""
            cs = slice(tch * QCH, (tch + 1) * QCH)
            swp = swpp.tile([128, QCH], BF16, tag="swp", name=f"swp{nm}")
            for h in range(2):
                o = h * 64
                nc.gpsimd.dma_start(swp[o:o + 32, :], zc[o + 32:o + 64, :])
                nc.gpsimd.dma_start(swp[o + 32:o + 64, :], zc[o:o + 32, :])
            tmp = rtp.tile([128, QCH], BF16, tag="rt", name=f"rt{nm}")
            nc.vector.tensor_mul(tmp[:], swp[:], sin_sb[:, cs])
            nc.vector.tensor_mul(zc[:], zc[:], cos_sb[:, cs])
            nc.vector.tensor_add(zc[:], zc[:], tmp[:])

        def oproj_unit(b, tt, evict_eng):
            yb = ybp.tile([128, C], BF16, tag="yb", name=f"y{b}{tt}")
            for ncol in range(2):
                ps = auxp.tile([128, QCH], F32, tag="aux", name=f"y{b}{tt}{ncol}")
                nc.tensor.matmul(
                    ps[:], OT[:, b, tt * 128:(tt + 1) * 128],
                    wo_sb[:, ncol * QCH:(ncol + 1) * QCH],
                    start=True, stop=True)
                dst = yb[:, ncol * QCH:(ncol + 1) * QCH]
                eng = evict_eng if ncol == 0 else "vector"
                if eng == "scalar":
                    nc.scalar.activation(dst, ps[:], AF.Copy)
                else:
                    nc.vector.tensor_copy(dst, ps[:])
            nc.gpsimd.dma_start(
                y[b * T + tt * 128:b * T + (tt + 1) * 128, :], yb[:])

        # ---- attention ----
        def attn_b(b, fillers, carry_in=None, tail_hook=None):
            carry = carry_in
            for qc in range(NQC):
                nk = 4 * (qc + 1) if mode == "causal" else NKT
                qs0 = b * T + qc * QCH
                # pso allocated lazily (after the carried tail's ptr, which
                # shares the pso0 buffer) to keep the buffer cycle acyclic
                pso = []
                pts = {}

                def qt_lo(kt, qc=qc):
                    return max(0, kt - 4 * qc) if mode == "causal" else 0

                def emit_pv(kt, qc=qc, pso=pso, pts=pts, nk=nk):
                    pt = pts.pop(kt)
                    for h in range(2):
                        for qtl in range(qt_lo(kt), 4):
                            qtg = 4 * qc + qtl
                            stop = (kt == qtg) if mode == "causal" \
                                else (kt == NKT - 1)
                            nc.tensor.matmul(
                                pso[h][:, qtl, 0:VW],
                                pt[:, h * QCH + qtl * 128:
                                   h * QCH + (qtl + 1) * 128],
                                Vsb[:, b, kt, h * VW:(h + 1) * VW],
                                start=(kt == 0 and qtl == 0), stop=stop,
                                skip_group_check=True)

                for kt in range(nk):
                    kk = slice((kt % 4) * 128, (kt % 4 + 1) * 128)
                    off = max(0, kt * 128 - qc * QCH) if mode == "causal" else 0
                    psS = psSp.tile([128, 2 * QCH], F32, tag="s",
                                    name=f"psS{b}{qc}{kt}")
                    for h in range(2):
                        nc.tensor.matmul(
                            psS[:, h * QCH + off:(h + 1) * QCH],
                            kTc[b][kt // 4][h * 64:(h + 1) * 64, kk],
                            qTc[b][qc][h * 64:(h + 1) * 64, off:QCH],
                            start=True, stop=True)
                    pt = ptp.tile([128, 2 * QCH], BF16, tag="pt",
                                  name=f"pt{b}{qc}{kt}")
                    if off == 0:
                        nc.scalar.activation(pt[:], psS[:], AF.Exp)
                    else:
                        pv = pt[:].rearrange("p (h q) -> p h q", h=2)
                        sv = psS[:].rearrange("p (h q) -> p h q", h=2)
                        nc.scalar.activation(pv[:, :, off:QCH],
                                             sv[:, :, off:QCH], AF.Exp)
                    if mode == "causal" and kt >= 4 * qc:
                        dq = kt - 4 * qc
                        blk = pt[:].rearrange("p (h q) -> p h q", h=2)[
                            :, :, dq * 128:(dq + 1) * 128]
                        nc.gpsimd.affine_select(
                            out=blk, in_=blk,
                            compare_op=mybir.AluOpType.is_ge, fill=0.0,
                            base=0, channel_multiplier=-1,
                            pattern=[[0, 2], [1, 128]])
                    elif mode == "bias":
                        mt = mbp.tile([128, QCH], F32, tag="mb",
                                      name=f"mt{b}{qc}{kt}")
                        nc.sync.dma_start(
                            mt[:], maskT[kt * 128:(kt + 1) * 128,
                                         qc * QCH:(qc + 1) * QCH])
                        nc.vector.tensor_mul(pt[:, 0:QCH], pt[:, 0:QCH], mt[:])
                        nc.vector.tensor_mul(pt[:, QCH:2 * QCH],
                                             pt[:, QCH:2 * QCH], mt[:])
                    pts[kt] = pt
                    if kt == 1:
                        if carry is not None:
                            carry()
                            carry = None
                        for h in (0, 1):
                            pso.append(psOp.tile([128, 4, 128], F32,
                                                 tag=f"pso{h}",
                                                 name=f"pso{b}{qc}{h}"))
                    if fillers:
                        fillers.popleft()()
                        if len(fillers) > 12:
                            fillers.popleft()()
                    if kt >= PIPE:
                        emit_pv(kt - PIPE)
                for j in range(max(0, nk - PIPE), nk):
                    emit_pv(j)

                # tail: normalize, transpose, evict to OT — deferred into the
                # next qc's pipeline so the PE never stalls on the norm chain
                def tail(qc=qc, pso=pso, b=b):
                    zsb = smol.tile([128, 8], F32, tag="z", name=f"z{b}{qc}")
                    for h in range(2):
                        nc.vector.tensor_copy(
                            zsb[:].rearrange("p (h q) -> p h q", h=2)[:, h, :],
                            pso[h][:, :, 0:1].rearrange("p a b -> p (a b)"))
                    rz = smol.tile([128, 8], F32, tag="rz", name=f"rz{b}{qc}")
                    nc.vector.reciprocal_approx_fast(rz[:], zsb[:])
                    ptr = psOp.tile([128, QCH], F32, tag="pso0",
                                    name=f"tr{b}{qc}")
                    for qtl in range(4):
                        on = onp.tile([128, 128], F32, tag="on",
                                      name=f"on{b}{qc}{qtl}")
                        for h in range(2):
                            nc.vector.tensor_scalar_mul(
                                on[:, h * 64:(h + 1) * 64],
                                pso[h][:, qtl, 1:65],
                                rz[:, h * 4 + qtl:h * 4 + qtl + 1])
                        nc.tensor.matmul(ptr[:, qtl * 128:(qtl + 1) * 128],
                                         on[:], id128[:], is_transpose=True,
                                         start=(qtl == 0), stop=True,
                                         skip_group_check=True)
                    nc.vector.tensor_copy(
                        OT[:, b, qc * QCH:(qc + 1) * QCH], ptr[:])
                    if tail_hook is not None:
                        tail_hook(qc)
                carry = tail
            return carry

        # ---- program ----
        from collections import deque
        # P1 head: only chunk 0 of batch 0, so attention starts immediately
        qk_unit(0, 0, 0, "scalar")
        qk_unit(0, 0, 1, "scalar")
        rope_unit(0, qTc[0][0], 0, "q00")
        rope_unit(0, kTc[0][0], 0, "k00")
        for vt in range(4):
            v_unit(0, vt)
        # P2: attn b0; fillers = rest of qkv b0 (per-chunk, just ahead of the
        # q-chunk that consumes it) then qkv b1
        f2 = deque()
        for tch in range(1, NQC):
            f2.append(lambda tch=tch: qk_unit(0, tch, 0, "vector"))
            f2.append(lambda tch=tch: qk_unit(0, tch, 1, "vector"))
            f2.append(lambda tch=tch: rope_unit(0, qTc[0][tch], tch,
                                                f"q0{tch}"))
            f2.append(lambda tch=tch: rope_unit(0, kTc[0][tch], tch,
                                                f"k0{tch}"))
            for vt in range(4 * tch, 4 * tch + 4):
                f2.append(lambda vt=vt: v_unit(0, vt))
        f2.appendleft(lambda: load_unit(4))
        f2.insert(4, lambda: load_unit(5))
        f2.insert(8, lambda: load_unit(6))
        f2.insert(12, lambda: load_unit(7))
        f2.insert(16, lambda: load_unit(8))
        for tch in range(NQC):
            f2.append(lambda tch=tch: qk_unit(1, tch, 0, "vector"))
            f2.append(lambda tch=tch: qk_unit(1, tch, 1, "vector"))
            f2.append(lambda tch=tch: rope_unit(1, qTc[1][tch], tch,
                                                f"q1{tch}"))
            f2.append(lambda tch=tch: rope_unit(1, kTc[1][tch], tch,
                                                f"k1{tch}"))
        for vt in range(NKT):
            f2.append(lambda vt=vt: v_unit(1, vt))
        carry = attn_b(0, f2)
        while f2:
            f2.popleft()()
        # P3: attn b1, fillers = out-proj b0 (+ out-proj b1 streamed in as
        # each b1 q-chunk's OT becomes ready)
        f3 = deque(lambda tt=tt: oproj_unit(0, tt, "vector")
                   for tt in range(NKT))

        def b1_tail_hook(qc):
            if qc < NQC - 1:
                for tt in range(4 * qc, 4 * qc + 4):
                    f3.append(lambda tt=tt: oproj_unit(1, tt, "vector"))

        carry = attn_b(1, f3, carry_in=carry, tail_hook=b1_tail_hook)
        while f3:
            f3.popleft()()
        # P4: finish b1 tail + remaining out-proj b1
        carry()
        for tt in range(4 * (NQC - 1), NKT):
            oproj_unit(1, tt, "scalar")


def _build_program(mode):
    if mode in _PROG_CACHE:
        return _PROG_CACHE[mode]
    nc = bacc.Bacc("TRN2", target_bir_lowering=False, debug=False,
                   num_devices=NCORES)
    dram = {
        "xT": nc.dram_tensor("xT", [128, 8, NCT, 512], BF16,
                             kind="ExternalInput").ap(),
        "wqkT": nc.dram_tensor("wqkT", [128, NCT, 256], BF16,
                               kind="ExternalInput").ap(),
        "wvT": nc.dram_tensor("wvT", [128, NCT, 128], BF16,
                              kind="ExternalInput").ap(),
        "bqk": nc.dram_tensor("bqk", [128, 2], F32, kind="ExternalInput").ap(),
        "cosT": nc.dram_tensor("cosT", [128, T], BF16,
                               kind="ExternalInput").ap(),
        "sinS": nc.dram_tensor("sinS", [128, T], BF16,
                               kind="ExternalInput").ap(),
        "woT": nc.dram_tensor("woT", [128, C], BF16,
                              kind="ExternalInput").ap(),
        "y": nc.dram_tensor("y", [TB, C], BF16, kind="ExternalOutput").ap(),
    }
    if mode == "bias":
        dram["maskT"] = nc.dram_tensor("maskT", [T, T], F32,
                                       kind="ExternalInput").ap()
    with tile.TileContext(nc) as tc:
        _emit(tc, mode, dram)
    nc.compile()
    _PROG_CACHE[mode] = (nc, dram)
    return nc, dram


def _rope_tables():
    inv_freq = 1.0 / (10000.0 ** (np.arange(0, HD, 2, dtype=np.float64) / HD))
    freqs = np.arange(T, dtype=np.float64)[:, None] * inv_freq[None, :]
    cos = np.concatenate([np.cos(freqs), np.cos(freqs)], axis=-1)  # [T, 64]
    sin = np.concatenate([np.sin(freqs), np.sin(freqs)], axis=-1)
    cE = cos[:, 0::2].T  # [32, T] rows i -> dim 2i
    cO = cos[:, 1::2].T
    sE = sin[:, 0::2].T
    sO = sin[:, 1::2].T
    cosT = np.concatenate([cE, cO, cE, cO], axis=0)
    sinS = np.concatenate([-sE, sO, -sE, sO], axis=0)
    return (np.ascontiguousarray(cosT.astype(BF)),
            np.ascontiguousarray(sinS.astype(BF)))


def _detect_mode(mask):
    mb = mask.reshape(T, T)
    if np.array_equal(mb != 0, np.tril(np.ones((T, T), dtype=bool))):
        return "causal", mb
    if np.all(mb != 0):
        return "dense", mb
    return "bias", mb


def _prepare_in_maps(x, mask, Wqkv, bqkv, Wo, bo, mode, mb):
    x = np.asarray(x, dtype=np.float32)
    Wqkv = np.asarray(Wqkv, dtype=np.float32)
    bqkv = np.asarray(bqkv, dtype=np.float32)
    Wo = np.asarray(Wo, dtype=np.float32)

    # pre-tiled x: [p, chunk, ct, m] with value xT[ct*128+p, chunk*512+m]
    xTf = x.reshape(TB, C).T.astype(BF)           # [C, TB]
    xTn = np.ascontiguousarray(
        xTf.reshape(NCT, 128, 8, 512).transpose(1, 2, 0, 3))
    cosT, sinS = _rope_tables()
    scale = 1.0 / np.sqrt(np.float32(HD))
    evens = np.arange(0, HD, 2)
    odds = evens + 1

    in_maps = []
    for c in range(NCORES):
        h0, h1 = 2 * c, 2 * c + 1
        qrows = np.concatenate([h0 * HD + evens, h0 * HD + odds,
                                h1 * HD + evens, h1 * HD + odds])
        krows = C + qrows
        vrows = np.concatenate([2 * C + h0 * HD + np.arange(HD),
                                2 * C + h1 * HD + np.arange(HD)])
        wq = Wqkv[qrows, :] * scale
        wk = Wqkv[krows, :]
        wv = Wqkv[vrows, :]
        # pre-tiled weights: [p, ct, outdim] with value W.T[ct*128+p, outdim]
        wqkT = np.ascontiguousarray(
            np.concatenate([wq, wk], axis=0).T.astype(BF)
            .reshape(NCT, 128, 256).transpose(1, 0, 2))
        wvT = np.ascontiguousarray(
            wv.T.astype(BF).reshape(NCT, 128, 128).transpose(1, 0, 2))
        bqk = np.stack([bqkv[qrows] * scale, bqkv[krows]], axis=1)
        woT = np.ascontiguousarray(Wo[:, 128 * c:128 * (c + 1)].T.astype(BF))
        im = {
            "xT": xTn, "wqkT": wqkT, "wvT": wvT,
            "bqk": np.ascontiguousarray(bqk, dtype=np.float32),
            "cosT": cosT, "sinS": sinS, "woT": woT,
        }
        if mode == "bias":
            im["maskT"] = np.ascontiguousarray(
                (mb != 0).astype(np.float32).T)
        in_maps.append(im)
    return in_maps


def kernel(x, mask, Wqkv, bqkv, Wo, bo):
    mask = np.asarray(mask)
    bqkv = np.asarray(bqkv, dtype=np.float32)
    Wo = np.asarray(Wo, dtype=np.float32)
    bo = np.asarray(bo, dtype=np.float32)

    mode, mb = _detect_mode(mask)
    nc, dram = _build_program(mode)
    in_maps = _prepare_in_maps(x, mask, Wqkv, bqkv, Wo, bo, mode, mb)

    res = run_bass_kernel_spmd(nc, in_maps, core_ids=list(range(NCORES)))
    y = np.zeros((TB, C), dtype=np.float32)
    for c in range(NCORES):
        y += np.asarray(res.results[c]["y"], dtype=np.float32)
    bv = bqkv[2 * C:3 * C]
    y += (bo + bv @ Wo.T)[None, :]
    return y.reshape(B, T, C)


# revision 30
# speedup vs baseline: 1.0934x; 1.0934x over previous
"""Trainium2 Bass kernel for CarlosSelfAttention (B=2, T=2048, C=1024, H=16).

Sharding: tensor-parallel over heads. 8 cores x 2 heads each. Each core
computes q/k/v projections for its 2 heads, RoPE, causal attention, and a
partial out-projection against its 128 columns of Wo. The host sums the 8
partial outputs (the TP all-reduce) and adds the output bias plus the
(v-bias @ Wo.T) correction term.

All-bf16 datapath (fp32 PSUM accumulation). Per-core layout:
  xsb  [128, 8ct, 4096]   whole input resident in SBUF, bf16
  qT/kT[128, 4096]        rows = [h0-even, h0-odd, h1-even, h1-odd] dims
  Vsb  [128, b, kt, 132]  V^T tiles: per head 66 cols = [ones, 64 dims, pad]
  S^T  [128 kpos, 1024]   psum; exp'd on ScalarE -> pt bf16, causal-trimmed
  PV   stationary = P^T [128,128] tile, moving = Vsb 66-wide; 8 accumulation
       groups packed in 2 psum banks (single bank reset at kt=0).
  norm 1/Z via per-partition tensor_scalar_mul; PE-transpose -> OT [dims,tok]
  out  y_part [4096, 1024] bf16 = OT.T @ WoT via PE.
QKV of batch 1 and out-proj of batch 0 run as fillers inside the attention
phases to keep the tensor engine dense.
"""

import numpy as np
import ml_dtypes

import concourse.bass as bass
import concourse.tile as tile
from concourse import bacc, mybir
from concourse.bass_utils import run_bass_kernel_spmd

F32 = mybir.dt.float32
BF16 = mybir.dt.bfloat16
AF = mybir.ActivationFunctionType
BF = ml_dtypes.bfloat16

B, T, C, H, HD = 2, 2048, 1024, 16, 64
NCORES = 8
TB = B * T          # 4096
QCH = 512           # q-chunk
NQC = T // QCH      # 4 q-chunks per batch
NKT = T // 128      # 16 k-tiles per batch
NCT = C // 128      # 8 contraction tiles
VW = HD + 2         # 66: [ones, 64 dims, pad] moving width per head in PV
PIPE = 2

_PROG_CACHE: dict = {}


def _emit(tc, mode, dram):
    nc = tc.nc
    from contextlib import ExitStack

    xT, wqkT, wvT, bqk, cosT, sinS, woT, y = (
        dram["xT"], dram["wqkT"], dram["wvT"], dram["bqk"], dram["cosT"],
        dram["sinS"], dram["woT"], dram["y"])
    maskT = dram.get("maskT")

    with ExitStack() as ctx:
        constp = ctx.enter_context(tc.tile_pool(name="const", bufs=1))
        pers = ctx.enter_context(tc.tile_pool(name="pers", bufs=1))
        psSp = ctx.enter_context(tc.tile_pool(name="psS", bufs=2, space="PSUM"))
        psOp = ctx.enter_context(tc.tile_pool(name="psO", bufs=1, space="PSUM"))
        auxp = ctx.enter_context(tc.tile_pool(name="aux", bufs=2, space="PSUM"))
        ptp = ctx.enter_context(tc.tile_pool(name="ptp", bufs=5))
        swpp = ctx.enter_context(tc.tile_pool(name="swp", bufs=2))
        rtp = ctx.enter_context(tc.tile_pool(name="rtp", bufs=2))
        onp = ctx.enter_context(tc.tile_pool(name="onp", bufs=6))
        smol = ctx.enter_context(tc.tile_pool(name="smol", bufs=4))
        ybp = ctx.enter_context(tc.tile_pool(name="ybp", bufs=4))
        mbp = ctx.enter_context(tc.tile_pool(name="mbp", bufs=4))

        # ---- constants (all host-pre-tiled: contiguous DMAs) ----
        wqk_sb = constp.tile([128, NCT, 256], BF16)
        nc.sync.dma_start(wqk_sb[:], wqkT[:])
        bqk_sb = constp.tile([128, 2], F32)
        nc.sync.dma_start(bqk_sb[:], bqk[:])
        # whole input resident in SBUF, one tile per 512-token chunk so the
        # first projection unit only waits for its own chunk's DMA
        xch = [pers.tile([128, NCT, 512], BF16, name=f"xch{ch}")
               for ch in range(8)]
        for ch in (0, 1, 2, 3):
            nc.sync.dma_start(xch[ch][:], xT[:, ch, :, :])
        cos_sb = constp.tile([128, T], BF16)
        nc.sync.dma_start(cos_sb[:], cosT[:])
        sin_sb = constp.tile([128, T], BF16)
        nc.sync.dma_start(sin_sb[:], sinS[:])
        wv_sb = constp.tile([128, NCT, 128], BF16)
        nc.sync.dma_start(wv_sb[:], wvT[:])
        wo_sb = constp.tile([128, C], BF16)

        def load_unit(ch):
            if ch < 8:
                nc.sync.dma_start(xch[ch][:], xT[:, ch, :, :])
            else:
                nc.sync.dma_start(wo_sb[:], woT[:])

        def xs(b, tok0, width):
            """xsb slice [128, NCT, width] for batch b tokens [tok0, tok0+width)."""
            ch, o = divmod(b * T + tok0, 512)
            assert o + width <= 512
            return xch[ch][:, :, o:o + width]
        id128 = constp.tile([128, 128], F32)
        nc.vector.memset(id128[:], 1.0)
        nc.gpsimd.affine_select(
            out=id128[:], in_=id128[:], compare_op=mybir.AluOpType.is_equal,
            fill=0.0, base=0, channel_multiplier=1, pattern=[[-1, 128]])

        # ---- persistent activations (per-chunk tiles: fine-grained deps) ----
        qTc = [[pers.tile([128, QCH], BF16, name=f"qT{b}{tch}")
                for tch in range(NQC)] for b in range(B)]
        kTc = [[pers.tile([128, QCH], BF16, name=f"kT{b}{tch}")
                for tch in range(NQC)] for b in range(B)]
        # V^T tiles [128 kpos, B, NKT, 2*VW]; ones/pad cols preset to 1.0
        Vsb = pers.tile([128, B, NKT, 2 * VW], BF16)
        nc.vector.memset(Vsb[:], 1.0)
        OT = pers.tile([128, B, T], BF16)

        # ---- unit emitters ----
        def qk_unit(b, tch, g, evict_eng):
            """q or k projection for one 512-token chunk of batch b."""
            xv = xs(b, tch * QCH, QCH)
            ps = auxp.tile([128, QCH], F32, tag="aux", name=f"qk{b}{tch}{g}")
            for ct in range(NCT):
                nc.tensor.matmul(ps[:], wqk_sb[:, ct, g * 128:(g + 1) * 128],
                                 xv[:, ct, :], start=(ct == 0),
                                 stop=(ct == NCT - 1))
            dst = (qTc if g == 0 else kTc)[b][tch][:]
            if evict_eng == "scalar":
                nc.scalar.activation(dst, ps[:], AF.Identity,
                                     bias=bqk_sb[:, g:g + 1])
            else:
                nc.vector.tensor_scalar_add(dst, ps[:], bqk_sb[:, g:g + 1])

        def v_unit(b, vt):
            """V^T for one 128-token tile of batch b (both heads)."""
            xv = xs(b, vt * 128, 128)
            ps = auxp.tile([128, QCH], F32, tag="aux", name=f"v{b}{vt}")
            for ct in range(NCT):
                nc.tensor.matmul(ps[:, 0:128], xv[:, ct, :],
                                 wv_sb[:, ct, :], start=(ct == 0),
                                 stop=(ct == NCT - 1))
            dst = Vsb[:, b, vt, :].rearrange("p (h c) -> p h c", h=2)[:, :, 1:65]
            nc.vector.tensor_copy(
                dst, ps[:, 0:128].rearrange("p (h c) -> p h c", h=2))

        def rope_unit(b, zc, tch, nm):
            """RoPE in-place on one per-chunk tile zc = (qTc|kTc)[b][tch]."""
            cs = slice(tch * QCH, (tch + 1) * QCH)
            swp = swpp.tile([128, QCH], BF16, tag="swp", name=f"swp{nm}")
            for h in range(2):
                o = h * 64
                nc.sync.dma_start(swp[o:o + 32, :], zc[o + 32:o + 64, :])
                nc.sync.dma_start(swp[o + 32:o + 64, :], zc[o:o + 32, :])
            tmp = rtp.tile([128, QCH], BF16, tag="rt", name=f"rt{nm}")
            nc.vector.tensor_mul(tmp[:], swp[:], sin_sb[:, cs])
            nc.vector.tensor_mul(zc[:], zc[:], cos_sb[:, cs])
            nc.vector.tensor_add(zc[:], zc[:], tmp[:])

        def oproj_unit(b, tt, evict_eng):
            yb = ybp.tile([128, C], BF16, tag="yb", name=f"y{b}{tt}")
            for ncol in range(2):
                ps = auxp.tile([128, QCH], F32, tag="aux", name=f"y{b}{tt}{ncol}")
                nc.tensor.matmul(
                    ps[:], OT[:, b, tt * 128:(tt + 1) * 128],
                    wo_sb[:, ncol * QCH:(ncol + 1) * QCH],
                    start=True, stop=True)
                dst = yb[:, ncol * QCH:(ncol + 1) * QCH]
                eng = evict_eng if ncol == 0 else "vector"
                if eng == "scalar":
                    nc.scalar.activation(dst, ps[:], AF.Copy)
                else:
                    nc.vector.tensor_copy(dst, ps[:])
            nc.sync.dma_start(
                y[b * T + tt * 128:b * T + (tt + 1) * 128, :], yb[:])

        # ---- attention ----
        def attn_b(b, fillers, carry_in=None, tail_hook=None):
            carry = carry_in
            for qc in range(NQC):
                nk = 4 * (qc + 1) if mode == "causal" else NKT
                qs0 = b * T + qc * QCH
                # pso allocated lazily (after the carried tail's ptr, which
                # shares the pso0 buffer) to keep the buffer cycle acyclic
                pso = []
                pts = {}

                def qt_lo(kt, qc=qc):
                    return max(0, kt - 4 * qc) if mode == "causal" else 0

                def emit_pv(kt, qc=qc, pso=pso, pts=pts, nk=nk):
                    pt = pts.pop(kt)
                    for h in range(2):
                        for qtl in range(qt_lo(kt), 4):
                            qtg = 4 * qc + qtl
                            stop = (kt == qtg) if mode == "causal" \
                                else (kt == NKT - 1)
                            nc.tensor.matmul(
                                pso[h][:, qtl, 0:VW],
                                pt[:, h * QCH + qtl * 128:
                                   h * QCH + (qtl + 1) * 128],
                                Vsb[:, b, kt, h * VW:(h + 1) * VW],
                                start=(kt == 0 and qtl == 0), stop=stop,
                                skip_group_check=True)

                for kt in range(nk):
                    kk = slice((kt % 4) * 128, (kt % 4 + 1) * 128)
                    off = max(0, kt * 128 - qc * QCH) if mode == "causal" else 0
                    psS = psSp.tile([128, 2 * QCH], F32, tag="s",
                                    name=f"psS{b}{qc}{kt}")
                    for h in range(2):
                        nc.tensor.matmul(
                            psS[:, h * QCH + off:(h + 1) * QCH],
                            kTc[b][kt // 4][h * 64:(h + 1) * 64, kk],
                            qTc[b][qc][h * 64:(h + 1) * 64, off:QCH],
                            start=True, stop=True)
                    pt = ptp.tile([128, 2 * QCH], BF16, tag="pt",
                                  name=f"pt{b}{qc}{kt}")
                    if off == 0:
                        nc.scalar.activation(pt[:], psS[:], AF.Exp)
                    else:
                        pv = pt[:].rearrange("p (h q) -> p h q", h=2)
                        sv = psS[:].rearrange("p (h q) -> p h q", h=2)
                        nc.scalar.activation(pv[:, :, off:QCH],
                                             sv[:, :, off:QCH], AF.Exp)
                    if mode == "causal" and kt >= 4 * qc:
                        dq = kt - 4 * qc
                        blk = pt[:].rearrange("p (h q) -> p h q", h=2)[
                            :, :, dq * 128:(dq + 1) * 128]
                        nc.gpsimd.affine_select(
                            out=blk, in_=blk,
                            compare_op=mybir.AluOpType.is_ge, fill=0.0,
                            base=0, channel_multiplier=-1,
                            pattern=[[0, 2], [1, 128]])
                    elif mode == "bias":
                        mt = mbp.tile([128, QCH], F32, tag="mb",
                                      name=f"mt{b}{qc}{kt}")
                        nc.sync.dma_start(
                            mt[:], maskT[kt * 128:(kt + 1) * 128,
                                         qc * QCH:(qc + 1) * QCH])
                        nc.vector.tensor_mul(pt[:, 0:QCH], pt[:, 0:QCH], mt[:])
                        nc.vector.tensor_mul(pt[:, QCH:2 * QCH],
                                             pt[:, QCH:2 * QCH], mt[:])
                    pts[kt] = pt
                    if kt == 1:
                        if carry is not None:
                            carry()
                            carry = None
                        for h in (0, 1):
                            pso.append(psOp.tile([128, 4, 128], F32,
                                                 tag=f"pso{h}",
                                                 name=f"pso{b}{qc}{h}"))
                    if fillers:
                        fillers.popleft()()
                        if len(fillers) > 12:
                            fillers.popleft()()
                    if kt >= PIPE:
                        emit_pv(kt - PIPE)
                for j in range(max(0, nk - PIPE), nk):
                    emit_pv(j)

                # tail: normalize, transpose, evict to OT — deferred into the
                # next qc's pipeline so the PE never stalls on the norm chain
                def tail(qc=qc, pso=pso, b=b):
                    zsb = smol.tile([128, 8], F32, tag="z", name=f"z{b}{qc}")
                    for h in range(2):
                        nc.vector.tensor_copy(
                            zsb[:].rearrange("p (h q) -> p h q", h=2)[:, h, :],
                            pso[h][:, :, 0:1].rearrange("p a b -> p (a b)"))
                    rz = smol.tile([128, 8], F32, tag="rz", name=f"rz{b}{qc}")
                    nc.vector.reciprocal_approx_fast(rz[:], zsb[:])
                    ptr = psOp.tile([128, QCH], F32, tag="pso0",
                                    name=f"tr{b}{qc}")
                    for qtl in range(4):
                        on = onp.tile([128, 128], F32, tag="on",
                                      name=f"on{b}{qc}{qtl}")
                        for h in range(2):
                            nc.vector.tensor_scalar_mul(
                                on[:, h * 64:(h + 1) * 64],
                                pso[h][:, qtl, 1:65],
                                rz[:, h * 4 + qtl:h * 4 + qtl + 1])
                        nc.tensor.matmul(ptr[:, qtl * 128:(qtl + 1) * 128],
                                         on[:], id128[:], is_transpose=True,
                                         start=(qtl == 0), stop=True,
                                         skip_group_check=True)
                    nc.vector.tensor_copy(
                        OT[:, b, qc * QCH:(qc + 1) * QCH], ptr[:])
                    if tail_hook is not None:
                        tail_hook(qc)
                carry = tail
            return carry

        # ---- program ----
        from collections import deque
        # P1 head: only chunk 0 of batch 0, so attention starts immediately
        qk_unit(0, 0, 0, "scalar")
        qk_unit(0, 0, 1, "scalar")
        rope_unit(0, qTc[0][0], 0, "q00")
        rope_unit(0, kTc[0][0], 0, "k00")
        for vt in range(4):
            v_unit(0, vt)
        # P2: attn b0; fillers = rest of qkv b0 (per-chunk, just ahead of the
        # q-chunk that consumes it) then qkv b1
        f2 = deque()
        for tch in range(1, NQC):
            f2.append(lambda tch=tch: qk_unit(0, tch, 0, "vector"))
            f2.append(lambda tch=tch: qk_unit(0, tch, 1, "vector"))
            f2.append(lambda tch=tch: rope_unit(0, qTc[0][tch], tch,
                                                f"q0{tch}"))
            f2.append(lambda tch=tch: rope_unit(0, kTc[0][tch], tch,
                                                f"k0{tch}"))
            for vt in range(4 * tch, 4 * tch + 4):
                f2.append(lambda vt=vt: v_unit(0, vt))
        f2.appendleft(lambda: load_unit(4))
        f2.insert(4, lambda: load_unit(5))
        f2.insert(8, lambda: load_unit(6))
        f2.insert(12, lambda: load_unit(7))
        f2.insert(16, lambda: load_unit(8))
        for tch in range(NQC):
            f2.append(lambda tch=tch: qk_unit(1, tch, 0, "vector"))
            f2.append(lambda tch=tch: qk_unit(1, tch, 1, "vector"))
            f2.append(lambda tch=tch: rope_unit(1, qTc[1][tch], tch,
                                                f"q1{tch}"))
            f2.append(lambda tch=tch: rope_unit(1, kTc[1][tch], tch,
                                                f"k1{tch}"))
        for vt in range(NKT):
            f2.append(lambda vt=vt: v_unit(1, vt))
        carry = attn_b(0, f2)
        while f2:
            f2.popleft()()
        # P3: attn b1, fillers = out-proj b0 (+ out-proj b1 streamed in as
        # each b1 q-chunk's OT becomes ready)
        f3 = deque(lambda tt=tt: oproj_unit(0, tt, "vector")
                   for tt in range(NKT))

        def b1_tail_hook(qc):
            if qc < NQC - 1:
                for tt in range(4 * qc, 4 * qc + 4):
                    f3.append(lambda tt=tt: oproj_unit(1, tt, "vector"))

        carry = attn_b(1, f3, carry_in=carry, tail_hook=b1_tail_hook)
        while f3:
            f3.popleft()()
        # P4: finish b1 tail + remaining out-proj b1
        carry()
        for tt in range(4 * (NQC - 1), NKT):
            oproj_unit(1, tt, "scalar")


def _build_program(mode):
    if mode in _PROG_CACHE:
        return _PROG_CACHE[mode]
    nc = bacc.Bacc("TRN2", target_bir_lowering=False, debug=False,
                   num_devices=NCORES)
    dram = {
        "xT": nc.dram_tensor("xT", [128, 8, NCT, 512], BF16,
                             kind="ExternalInput").ap(),
        "wqkT": nc.dram_tensor("wqkT", [128, NCT, 256], BF16,
                               kind="ExternalInput").ap(),
        "wvT": nc.dram_tensor("wvT", [128, NCT, 128], BF16,
                              kind="ExternalInput").ap(),
        "bqk": nc.dram_tensor("bqk", [128, 2], F32, kind="ExternalInput").ap(),
        "cosT": nc.dram_tensor("cosT", [128, T], BF16,
                               kind="ExternalInput").ap(),
        "sinS": nc.dram_tensor("sinS", [128, T], BF16,
                               kind="ExternalInput").ap(),
        "woT": nc.dram_tensor("woT", [128, C], BF16,
                              kind="ExternalInput").ap(),
        "y": nc.dram_tensor("y", [TB, C], BF16, kind="ExternalOutput").ap(),
    }
    if mode == "bias":
        dram["maskT"] = nc.dram_tensor("maskT", [T, T], F32,
                                       kind="ExternalInput").ap()
    with tile.TileContext(nc) as tc:
        _emit(tc, mode, dram)
    nc.compile()
    _PROG_CACHE[mode] = (nc, dram)
    return nc, dram


def _rope_tables():
    inv_freq = 1.0 / (10000.0 ** (np.arange(0, HD, 2, dtype=np.float64) / HD))
    freqs = np.arange(T, dtype=np.float64)[:, None] * inv_freq[None, :]
    cos = np.concatenate([np.cos(freqs), np.cos(freqs)], axis=-1)  # [T, 64]
    sin = np.concatenate([np.sin(freqs), np.sin(freqs)], axis=-1)
    cE = cos[:, 0::2].T  # [32, T] rows i -> dim 2i
    cO = cos[:, 1::2].T
    sE = sin[:, 0::2].T
    sO = sin[:, 1::2].T
    cosT = np.concatenate([cE, cO, cE, cO], axis=0)
    sinS = np.concatenate([-sE, sO, -sE, sO], axis=0)
    return (np.ascontiguousarray(cosT.astype(BF)),
            np.ascontiguousarray(sinS.astype(BF)))


def _detect_mode(mask):
    mb = mask.reshape(T, T)
    if np.array_equal(mb != 0, np.tril(np.ones((T, T), dtype=bool))):
        return "causal", mb
    if np.all(mb != 0):
        return "dense", mb
    return "bias", mb


def _prepare_in_maps(x, mask, Wqkv, bqkv, Wo, bo, mode, mb):
    x = np.asarray(x, dtype=np.float32)
    Wqkv = np.asarray(Wqkv, dtype=np.float32)
    bqkv = np.asarray(bqkv, dtype=np.float32)
    Wo = np.asarray(Wo, dtype=np.float32)

    # pre-tiled x: [p, chunk, ct, m] with value xT[ct*128+p, chunk*512+m]
    xTf = x.reshape(TB, C).T.astype(BF)           # [C, TB]
    xTn = np.ascontiguousarray(
        xTf.reshape(NCT, 128, 8, 512).transpose(1, 2, 0, 3))
    cosT, sinS = _rope_tables()
    scale = 1.0 / np.sqrt(np.float32(HD))
    evens = np.arange(0, HD, 2)
    odds = evens + 1

    in_maps = []
    for c in range(NCORES):
        h0, h1 = 2 * c, 2 * c + 1
        qrows = np.concatenate([h0 * HD + evens, h0 * HD + odds,
                                h1 * HD + evens, h1 * HD + odds])
        krows = C + qrows
        vrows = np.concatenate([2 * C + h0 * HD + np.arange(HD),
                                2 * C + h1 * HD + np.arange(HD)])
        wq = Wqkv[qrows, :] * scale
        wk = Wqkv[krows, :]
        wv = Wqkv[vrows, :]
        # pre-tiled weights: [p, ct, outdim] with value W.T[ct*128+p, outdim]
        wqkT = np.ascontiguousarray(
            np.concatenate([wq, wk], axis=0).T.astype(BF)
            .reshape(NCT, 128, 256).transpose(1, 0, 2))
        wvT = np.ascontiguousarray(
            wv.T.astype(BF).reshape(NCT, 128, 128).transpose(1, 0, 2))
        bqk = np.stack([bqkv[qrows] * scale, bqkv[krows]], axis=1)
        woT = np.ascontiguousarray(Wo[:, 128 * c:128 * (c + 1)].T.astype(BF))
        im = {
            "xT": xTn, "wqkT": wqkT, "wvT": wvT,
            "bqk": np.ascontiguousarray(bqk, dtype=np.float32),
            "cosT": cosT, "sinS": sinS, "woT": woT,
        }
        if mode == "bias":
            im["maskT"] = np.ascontiguousarray(
                (mb != 0).astype(np.float32).T)
        in_maps.append(im)
    return in_maps


def kernel(x, mask, Wqkv, bqkv, Wo, bo):
    mask = np.asarray(mask)
    bqkv = np.asarray(bqkv, dtype=np.float32)
    Wo = np.asarray(Wo, dtype=np.float32)
    bo = np.asarray(bo, dtype=np.float32)

    mode, mb = _detect_mode(mask)
    nc, dram = _build_program(mode)
    in_maps = _prepare_in_maps(x, mask, Wqkv, bqkv, Wo, bo, mode, mb)

    res = run_bass_kernel_spmd(nc, in_maps, core_ids=list(range(NCORES)))
    y = np.zeros((TB, C), dtype=np.float32)
    for c in range(NCORES):
        y += np.asarray(res.results[c]["y"], dtype=np.float32)
    bv = bqkv[2 * C:3 * C]
    y += (bo + bv @ Wo.T)[None, :]
    return y.reshape(B, T, C)


# revision 32
# speedup vs baseline: 1.1120x; 1.0169x over previous
"""Trainium2 Bass kernel for CarlosSelfAttention (B=2, T=2048, C=1024, H=16).

Sharding: tensor-parallel over heads. 8 cores x 2 heads each. Each core
computes q/k/v projections for its 2 heads, RoPE, causal attention, and a
partial out-projection against its 128 columns of Wo. The host sums the 8
partial outputs (the TP all-reduce) and adds the output bias plus the
(v-bias @ Wo.T) correction term.

All-bf16 datapath (fp32 PSUM accumulation). Per-core layout:
  xsb  [128, 8ct, 4096]   whole input resident in SBUF, bf16
  qT/kT[128, 4096]        rows = [h0-even, h0-odd, h1-even, h1-odd] dims
  Vsb  [128, b, kt, 132]  V^T tiles: per head 66 cols = [ones, 64 dims, pad]
  S^T  [128 kpos, 1024]   psum; exp'd on ScalarE -> pt bf16, causal-trimmed
  PV   stationary = P^T [128,128] tile, moving = Vsb 66-wide; 8 accumulation
       groups packed in 2 psum banks (single bank reset at kt=0).
  norm 1/Z via per-partition tensor_scalar_mul; PE-transpose -> OT [dims,tok]
  out  y_part [4096, 1024] bf16 = OT.T @ WoT via PE.
QKV of batch 1 and out-proj of batch 0 run as fillers inside the attention
phases to keep the tensor engine dense.
"""

import numpy as np
import ml_dtypes

import concourse.bass as bass
import concourse.tile as tile
from concourse import bacc, mybir
from concourse.bass_utils import run_bass_kernel_spmd

F32 = mybir.dt.float32
BF16 = mybir.dt.bfloat16
AF = mybir.ActivationFunctionType
BF = ml_dtypes.bfloat16

B, T, C, H, HD = 2, 2048, 1024, 16, 64
NCORES = 8
TB = B * T          # 4096
QCH = 512           # q-chunk
NQC = T // QCH      # 4 q-chunks per batch
NKT = T // 128      # 16 k-tiles per batch
NCT = C // 128      # 8 contraction tiles
VW = HD + 2         # 66: [ones, 64 dims, pad] moving width per head in PV
PIPE = 2

_PROG_CACHE: dict = {}


def _emit(tc, mode, dram):
    nc = tc.nc
    from contextlib import ExitStack

    xT, wqkT, wvT, bqk, cosT, sinS, woT, y = (
        dram["xT"], dram["wqkT"], dram["wvT"], dram["bqk"], dram["cosT"],
        dram["sinS"], dram["woT"], dram["y"])
    maskT = dram.get("maskT")

    with ExitStack() as ctx:
        constp = ctx.enter_context(tc.tile_pool(name="const", bufs=1))
        pers = ctx.enter_context(tc.tile_pool(name="pers", bufs=1))
        psSp = ctx.enter_context(tc.tile_pool(name="psS", bufs=2, space="PSUM"))
        psOp = ctx.enter_context(tc.tile_pool(name="psO", bufs=1, space="PSUM"))
        auxp = ctx.enter_context(tc.tile_pool(name="aux", bufs=2, space="PSUM"))
        ptp = ctx.enter_context(tc.tile_pool(name="ptp", bufs=5))
        swpp = ctx.enter_context(tc.tile_pool(name="swp", bufs=2))
        rtp = ctx.enter_context(tc.tile_pool(name="rtp", bufs=2))
        onp = ctx.enter_context(tc.tile_pool(name="onp", bufs=6))
        smol = ctx.enter_context(tc.tile_pool(name="smol", bufs=4))
        ybp = ctx.enter_context(tc.tile_pool(name="ybp", bufs=4))
        mbp = ctx.enter_context(tc.tile_pool(name="mbp", bufs=4))

        # ---- constants (all host-pre-tiled: contiguous DMAs) ----
        wqk_sb = constp.tile([128, NCT, 256], BF16)
        nc.sync.dma_start(wqk_sb[:], wqkT[:])
        bqk_sb = constp.tile([128, 2], F32)
        nc.sync.dma_start(bqk_sb[:], bqk[:])
        # whole input resident in SBUF, one tile per 512-token chunk so the
        # first projection unit only waits for its own chunk's DMA
        xch = [pers.tile([128, NCT, 512], BF16, name=f"xch{ch}")
               for ch in range(8)]
        nc.sync.dma_start(xch[0][:], xT[:, 0, :, :])
        cos_sb = constp.tile([128, T], BF16)
        nc.sync.dma_start(cos_sb[:], cosT[:])
        sin_sb = constp.tile([128, T], BF16)
        nc.sync.dma_start(sin_sb[:], sinS[:])
        wv_sb = constp.tile([128, NCT, 128], BF16)
        nc.sync.dma_start(wv_sb[:], wvT[:])
        wo_sb = constp.tile([128, C], BF16)

        def load_unit(ch):
            if ch < 8:
                nc.sync.dma_start(xch[ch][:], xT[:, ch, :, :])
            else:
                nc.sync.dma_start(wo_sb[:], woT[:])

        def xs(b, tok0, width):
            """xsb slice [128, NCT, width] for batch b tokens [tok0, tok0+width)."""
            ch, o = divmod(b * T + tok0, 512)
            assert o + width <= 512
            return xch[ch][:, :, o:o + width]
        id128 = constp.tile([128, 128], F32)
        nc.vector.memset(id128[:], 1.0)
        nc.gpsimd.affine_select(
            out=id128[:], in_=id128[:], compare_op=mybir.AluOpType.is_equal,
            fill=0.0, base=0, channel_multiplier=1, pattern=[[-1, 128]])

        # ---- persistent activations (per-chunk tiles: fine-grained deps) ----
        qT1 = pers.tile([128, T], BF16, name="qT1")
        kT1 = pers.tile([128, T], BF16, name="kT1")
        qTc = [[pers.tile([128, QCH], BF16, name=f"qT0{tch}")
                for tch in range(NQC)],
               [qT1[:, tch * QCH:(tch + 1) * QCH] for tch in range(NQC)]]
        kTc = [[pers.tile([128, QCH], BF16, name=f"kT0{tch}")
                for tch in range(NQC)],
               [kT1[:, tch * QCH:(tch + 1) * QCH] for tch in range(NQC)]]
        # V^T tiles [128 kpos, B, NKT, 2*VW]; ones/pad cols preset to 1.0
        Vsb = pers.tile([128, B, NKT, 2 * VW], BF16)
        nc.vector.memset(Vsb[:], 1.0)
        OT = pers.tile([128, B, T], BF16)

        # ---- unit emitters ----
        def qk_unit(b, tch, g, evict_eng):
            """q or k projection for one 512-token chunk of batch b."""
            xv = xs(b, tch * QCH, QCH)
            ps = auxp.tile([128, QCH], F32, tag="aux", name=f"qk{b}{tch}{g}")
            for ct in range(NCT):
                nc.tensor.matmul(ps[:], wqk_sb[:, ct, g * 128:(g + 1) * 128],
                                 xv[:, ct, :], start=(ct == 0),
                                 stop=(ct == NCT - 1))
            dst = (qTc if g == 0 else kTc)[b][tch][:]
            if evict_eng == "scalar":
                nc.scalar.activation(dst, ps[:], AF.Identity,
                                     bias=bqk_sb[:, g:g + 1])
            else:
                nc.vector.tensor_scalar_add(dst, ps[:], bqk_sb[:, g:g + 1])

        def v_unit(b, vt):
            """V^T for one 128-token tile of batch b (both heads)."""
            xv = xs(b, vt * 128, 128)
            ps = auxp.tile([128, QCH], F32, tag="aux", name=f"v{b}{vt}")
            for ct in range(NCT):
                nc.tensor.matmul(ps[:, 0:128], xv[:, ct, :],
                                 wv_sb[:, ct, :], start=(ct == 0),
                                 stop=(ct == NCT - 1))
            dst = Vsb[:, b, vt, :].rearrange("p (h c) -> p h c", h=2)[:, :, 1:65]
            nc.vector.tensor_copy(
                dst, ps[:, 0:128].rearrange("p (h c) -> p h c", h=2))

        def rope_unit(b, zc, tch, nm, dma_eng=None, width=QCH):
            """RoPE in-place on zc (a [128, width] tile/AP at chunk tch)."""
            cs = slice(tch * QCH, tch * QCH + width)
            de = dma_eng or nc.sync
            swp = swpp.tile([128, width], BF16, tag="swp", name=f"swp{nm}")
            for h in range(2):
                o = h * 64
                de.dma_start(swp[o:o + 32, :], zc[o + 32:o + 64, :])
                de.dma_start(swp[o + 32:o + 64, :], zc[o:o + 32, :])
            tmp = rtp.tile([128, width], BF16, tag="rt", name=f"rt{nm}")
            nc.vector.tensor_mul(tmp[:], swp[:], sin_sb[:, cs])
            nc.vector.tensor_mul(zc[:, 0:width], zc[:, 0:width], cos_sb[:, cs])
            nc.vector.tensor_add(zc[:, 0:width], zc[:, 0:width], tmp[:])

        def oproj_unit(b, tt, evict_eng):
            yb = ybp.tile([128, C], BF16, tag="yb", name=f"y{b}{tt}")
            for ncol in range(2):
                ps = auxp.tile([128, QCH], F32, tag="aux", name=f"y{b}{tt}{ncol}")
                nc.tensor.matmul(
                    ps[:], OT[:, b, tt * 128:(tt + 1) * 128],
                    wo_sb[:, ncol * QCH:(ncol + 1) * QCH],
                    start=True, stop=True)
                dst = yb[:, ncol * QCH:(ncol + 1) * QCH]
                eng = evict_eng if ncol == 0 else "vector"
                if eng == "scalar":
                    nc.scalar.activation(dst, ps[:], AF.Copy)
                else:
                    nc.vector.tensor_copy(dst, ps[:])
            nc.sync.dma_start(
                y[b * T + tt * 128:b * T + (tt + 1) * 128, :], yb[:])

        # ---- attention ----
        def attn_b(b, fillers, carry_in=None, tail_hook=None):
            carry = carry_in
            for qc in range(NQC):
                nk = 4 * (qc + 1) if mode == "causal" else NKT
                qs0 = b * T + qc * QCH
                # pso allocated lazily (after the carried tail's ptr, which
                # shares the pso0 buffer) to keep the buffer cycle acyclic
                pso = []
                pts = {}

                def qt_lo(kt, qc=qc):
                    return max(0, kt - 4 * qc) if mode == "causal" else 0

                def emit_pv(kt, qc=qc, pso=pso, pts=pts, nk=nk):
                    pt = pts.pop(kt)
                    for h in range(2):
                        for qtl in range(qt_lo(kt), 4):
                            qtg = 4 * qc + qtl
                            stop = (kt == qtg) if mode == "causal" \
                                else (kt == NKT - 1)
                            nc.tensor.matmul(
                                pso[h][:, qtl, 0:VW],
                                pt[:, h * QCH + qtl * 128:
                                   h * QCH + (qtl + 1) * 128],
                                Vsb[:, b, kt, h * VW:(h + 1) * VW],
                                start=(kt == 0 and qtl == 0), stop=stop,
                                skip_group_check=True)

                for kt in range(nk):
                    kk = slice((kt % 4) * 128, (kt % 4 + 1) * 128)
                    off = max(0, kt * 128 - qc * QCH) if mode == "causal" else 0
                    psS = psSp.tile([128, 2 * QCH], F32, tag="s",
                                    name=f"psS{b}{qc}{kt}")
                    for h in range(2):
                        nc.tensor.matmul(
                            psS[:, h * QCH + off:(h + 1) * QCH],
                            kTc[b][kt // 4][h * 64:(h + 1) * 64, kk],
                            qTc[b][qc][h * 64:(h + 1) * 64, off:QCH],
                            start=True, stop=True)
                    pt = ptp.tile([128, 2 * QCH], BF16, tag="pt",
                                  name=f"pt{b}{qc}{kt}")
                    if off == 0:
                        nc.scalar.activation(pt[:], psS[:], AF.Exp)
                    else:
                        pv = pt[:].rearrange("p (h q) -> p h q", h=2)
                        sv = psS[:].rearrange("p (h q) -> p h q", h=2)
                        nc.scalar.activation(pv[:, :, off:QCH],
                                             sv[:, :, off:QCH], AF.Exp)
                    if mode == "causal" and kt >= 4 * qc:
                        dq = kt - 4 * qc
                        blk = pt[:].rearrange("p (h q) -> p h q", h=2)[
                            :, :, dq * 128:(dq + 1) * 128]
                        nc.gpsimd.affine_select(
                            out=blk, in_=blk,
                            compare_op=mybir.AluOpType.is_ge, fill=0.0,
                            base=0, channel_multiplier=-1,
                            pattern=[[0, 2], [1, 128]])
                    elif mode == "bias":
                        mt = mbp.tile([128, QCH], F32, tag="mb",
                                      name=f"mt{b}{qc}{kt}")
                        nc.sync.dma_start(
                            mt[:], maskT[kt * 128:(kt + 1) * 128,
                                         qc * QCH:(qc + 1) * QCH])
                        nc.vector.tensor_mul(pt[:, 0:QCH], pt[:, 0:QCH], mt[:])
                        nc.vector.tensor_mul(pt[:, QCH:2 * QCH],
                                             pt[:, QCH:2 * QCH], mt[:])
                    pts[kt] = pt
                    if kt == 1:
                        if carry is not None:
                            carry()
                            carry = None
                        for h in (0, 1):
                            pso.append(psOp.tile([128, 4, 128], F32,
                                                 tag=f"pso{h}",
                                                 name=f"pso{b}{qc}{h}"))
                    if fillers:
                        fillers.popleft()()
                        if len(fillers) > 12:
                            fillers.popleft()()
                    if kt >= PIPE:
                        emit_pv(kt - PIPE)
                for j in range(max(0, nk - PIPE), nk):
                    emit_pv(j)

                # tail: normalize, transpose, evict to OT — deferred into the
                # next qc's pipeline so the PE never stalls on the norm chain
                def tail(qc=qc, pso=pso, b=b):
                    zsb = smol.tile([128, 8], F32, tag="z", name=f"z{b}{qc}")
                    for h in range(2):
                        nc.vector.tensor_copy(
                            zsb[:].rearrange("p (h q) -> p h q", h=2)[:, h, :],
                            pso[h][:, :, 0:1].rearrange("p a b -> p (a b)"))
                    rz = smol.tile([128, 8], F32, tag="rz", name=f"rz{b}{qc}")
                    nc.vector.reciprocal_approx_fast(rz[:], zsb[:])
                    ptr = psOp.tile([128, QCH], F32, tag="pso0",
                                    name=f"tr{b}{qc}")
                    for qtl in range(4):
                        on = onp.tile([128, 128], F32, tag="on",
                                      name=f"on{b}{qc}{qtl}")
                        for h in range(2):
                            nc.vector.tensor_scalar_mul(
                                on[:, h * 64:(h + 1) * 64],
                                pso[h][:, qtl, 1:65],
                                rz[:, h * 4 + qtl:h * 4 + qtl + 1])
                        nc.tensor.matmul(ptr[:, qtl * 128:(qtl + 1) * 128],
                                         on[:], id128[:], is_transpose=True,
                                         start=(qtl == 0), stop=True,
                                         skip_group_check=True)
                    nc.vector.tensor_copy(
                        OT[:, b, qc * QCH:(qc + 1) * QCH], ptr[:])
                    if tail_hook is not None:
                        tail_hook(qc)
                carry = tail
            return carry

        # ---- program ----
        from collections import deque
        # P1 head: only chunk 0 of batch 0, so attention starts immediately
        qk_unit(0, 0, 0, "scalar")
        qk_unit(0, 0, 1, "scalar")
        rope_unit(0, qTc[0][0], 0, "q00", dma_eng=nc.gpsimd)
        rope_unit(0, kTc[0][0], 0, "k00", dma_eng=nc.gpsimd)
        for vt in range(4):
            v_unit(0, vt)
        # P2: attn b0; fillers = rest of qkv b0 (per-chunk, just ahead of the
        # q-chunk that consumes it) then qkv b1
        f2 = deque()
        for tch in range(1, NQC):
            f2.append(lambda tch=tch: qk_unit(0, tch, 0, "vector"))
            f2.append(lambda tch=tch: qk_unit(0, tch, 1, "vector"))
            f2.append(lambda tch=tch: rope_unit(0, qTc[0][tch], tch,
                                                f"q0{tch}"))
            f2.append(lambda tch=tch: rope_unit(0, kTc[0][tch], tch,
                                                f"k0{tch}"))
            for vt in range(4 * tch, 4 * tch + 4):
                f2.append(lambda vt=vt: v_unit(0, vt))
        for pos, ch in ((0, 1), (2, 2), (5, 3), (9, 4), (13, 5),
                        (17, 6), (21, 7), (25, 8)):
            f2.insert(pos, lambda ch=ch: load_unit(ch))
        for tch in range(NQC):
            f2.append(lambda tch=tch: qk_unit(1, tch, 0, "vector"))
            f2.append(lambda tch=tch: qk_unit(1, tch, 1, "vector"))
        f2.append(lambda: rope_unit(1, qT1, 0, "q1", width=T))
        f2.append(lambda: rope_unit(1, kT1, 0, "k1", width=T))
        for vt in range(NKT):
            f2.append(lambda vt=vt: v_unit(1, vt))
        carry = attn_b(0, f2)
        while f2:
            f2.popleft()()
        # P3: attn b1, fillers = out-proj b0 (+ out-proj b1 streamed in as
        # each b1 q-chunk's OT becomes ready)
        f3 = deque(lambda tt=tt: oproj_unit(0, tt, "vector")
                   for tt in range(NKT))

        def b1_tail_hook(qc):
            if qc < NQC - 1:
                for tt in range(4 * qc, 4 * qc + 4):
                    f3.append(lambda tt=tt: oproj_unit(1, tt, "vector"))

        carry = attn_b(1, f3, carry_in=carry, tail_hook=b1_tail_hook)
        while f3:
            f3.popleft()()
        # P4: finish b1 tail + remaining out-proj b1
        carry()
        for tt in range(4 * (NQC - 1), NKT):
            oproj_unit(1, tt, "scalar")


def _build_program(mode):
    if mode in _PROG_CACHE:
        return _PROG_CACHE[mode]
    nc = bacc.Bacc("TRN2", target_bir_lowering=False, debug=False,
                   num_devices=NCORES)
    dram = {
        "xT": nc.dram_tensor("xT", [128, 8, NCT, 512], BF16,
                             kind="ExternalInput").ap(),
        "wqkT": nc.dram_tensor("wqkT", [128, NCT, 256], BF16,
                               kind="ExternalInput").ap(),
        "wvT": nc.dram_tensor("wvT", [128, NCT, 128], BF16,
                              kind="ExternalInput").ap(),
        "bqk": nc.dram_tensor("bqk", [128, 2], F32, kind="ExternalInput").ap(),
        "cosT": nc.dram_tensor("cosT", [128, T], BF16,
                               kind="ExternalInput").ap(),
        "sinS": nc.dram_tensor("sinS", [128, T], BF16,
                               kind="ExternalInput").ap(),
        "woT": nc.dram_tensor("woT", [128, C], BF16,
                              kind="ExternalInput").ap(),
        "y": nc.dram_tensor("y", [TB, C], BF16, kind="ExternalOutput").ap(),
    }
    if mode == "bias":
        dram["maskT"] = nc.dram_tensor("maskT", [T, T], F32,
                                       kind="ExternalInput").ap()
    with tile.TileContext(nc) as tc:
        _emit(tc, mode, dram)
    nc.compile()
    _PROG_CACHE[mode] = (nc, dram)
    return nc, dram


def _rope_tables():
    inv_freq = 1.0 / (10000.0 ** (np.arange(0, HD, 2, dtype=np.float64) / HD))
    freqs = np.arange(T, dtype=np.float64)[:, None] * inv_freq[None, :]
    cos = np.concatenate([np.cos(freqs), np.cos(freqs)], axis=-1)  # [T, 64]
    sin = np.concatenate([np.sin(freqs), np.sin(freqs)], axis=-1)
    cE = cos[:, 0::2].T  # [32, T] rows i -> dim 2i
    cO = cos[:, 1::2].T
    sE = sin[:, 0::2].T
    sO = sin[:, 1::2].T
    cosT = np.concatenate([cE, cO, cE, cO], axis=0)
    sinS = np.concatenate([-sE, sO, -sE, sO], axis=0)
    return (np.ascontiguousarray(cosT.astype(BF)),
            np.ascontiguousarray(sinS.astype(BF)))


def _detect_mode(mask):
    mb = mask.reshape(T, T)
    if np.array_equal(mb != 0, np.tril(np.ones((T, T), dtype=bool))):
        return "causal", mb
    if np.all(mb != 0):
        return "dense", mb
    return "bias", mb


def _prepare_in_maps(x, mask, Wqkv, bqkv, Wo, bo, mode, mb):
    x = np.asarray(x, dtype=np.float32)
    Wqkv = np.asarray(Wqkv, dtype=np.float32)
    bqkv = np.asarray(bqkv, dtype=np.float32)
    Wo = np.asarray(Wo, dtype=np.float32)

    # pre-tiled x: [p, chunk, ct, m] with value xT[ct*128+p, chunk*512+m]
    xTf = x.reshape(TB, C).T.astype(BF)           # [C, TB]
    xTn = np.ascontiguousarray(
        xTf.reshape(NCT, 128, 8, 512).transpose(1, 2, 0, 3))
    cosT, sinS = _rope_tables()
    scale = 1.0 / np.sqrt(np.float32(HD))
    evens = np.arange(0, HD, 2)
    odds = evens + 1

    in_maps = []
    for c in range(NCORES):
        h0, h1 = 2 * c, 2 * c + 1
        qrows = np.concatenate([h0 * HD + evens, h0 * HD + odds,
                                h1 * HD + evens, h1 * HD + odds])
        krows = C + qrows
        vrows = np.concatenate([2 * C + h0 * HD + np.arange(HD),
                                2 * C + h1 * HD + np.arange(HD)])
        wq = Wqkv[qrows, :] * scale
        wk = Wqkv[krows, :]
        wv = Wqkv[vrows, :]
        # pre-tiled weights: [p, ct, outdim] with value W.T[ct*128+p, outdim]
        wqkT = np.ascontiguousarray(
            np.concatenate([wq, wk], axis=0).T.astype(BF)
            .reshape(NCT, 128, 256).transpose(1, 0, 2))
        wvT = np.ascontiguousarray(
            wv.T.astype(BF).reshape(NCT, 128, 128).transpose(1, 0, 2))
        bqk = np.stack([bqkv[qrows] * scale, bqkv[krows]], axis=1)
        woT = np.ascontiguousarray(Wo[:, 128 * c:128 * (c + 1)].T.astype(BF))
        im = {
            "xT": xTn, "wqkT": wqkT, "wvT": wvT,
            "bqk": np.ascontiguousarray(bqk, dtype=np.float32),
            "cosT": cosT, "sinS": sinS, "woT": woT,
        }
        if mode == "bias":
            im["maskT"] = np.ascontiguousarray(
                (mb != 0).astype(np.float32).T)
        in_maps.append(im)
    return in_maps


def kernel(x, mask, Wqkv, bqkv, Wo, bo):
    mask = np.asarray(mask)
    bqkv = np.asarray(bqkv, dtype=np.float32)
    Wo = np.asarray(Wo, dtype=np.float32)
    bo = np.asarray(bo, dtype=np.float32)

    mode, mb = _detect_mode(mask)
    nc, dram = _build_program(mode)
    in_maps = _prepare_in_maps(x, mask, Wqkv, bqkv, Wo, bo, mode, mb)

    res = run_bass_kernel_spmd(nc, in_maps, core_ids=list(range(NCORES)))
    y = np.zeros((TB, C), dtype=np.float32)
    for c in range(NCORES):
        y += np.asarray(res.results[c]["y"], dtype=np.float32)
    bv = bqkv[2 * C:3 * C]
    y += (bo + bv @ Wo.T)[None, :]
    return y.reshape(B, T, C)
